# revision 9
# baseline (speedup 1.0000x reference)
"""Trainium2 Bass kernel for CMPNEncoder functional-group embedding.

Math: out = func_save_init + segment_sum(padded[func2atom].sum(1), mapping) @ ...
Reformulated: out = func_save_init + A @ W where
  A[m, :] = sum over all (fg, k) with mapping[fg] == m and func2atom[fg, k] > 0
            of f_atoms[func2atom[fg, k] - 1, :]
(matmul distributes over the gather+segment sums, so the [400k,133]@[133,300]
matmul collapses to a [100,133]@[133,300] one after the segment reduction).

Sharding: atoms are sharded 8 ways (50k rows per core).  Each core receives
the (atom, segment) pair list for its atom range, gathers the rows on-device
with dma_gather (768B padded rows), reduces them into a [100,133] PSUM
accumulator via one-hot matmuls on the tensor engine, applies W on-device,
and returns a partial [100,300].  Host sums the 8 partials (unshard) and
adds func_save_init.
"""

import sys

sys.path.insert(0, "/opt/trn_rl_repo")

import numpy as np

import concourse.bacc as bacc
import concourse.bass as bass
import concourse.mybir as mybir
from concourse.bass_utils import run_bass_kernel_spmd
from concourse.tile import TileContext

# Problem constants (hardcoded per the task contract).
N_ATOMS = 400_000
FDIM = 133
HID = 300
NSEG = 100
N_CORES = 8

PAD_F = 192          # f32 row padded to 768B (dma_gather needs 256B multiples)
ROWS_PER_CORE = N_ATOMS // N_CORES   # 50_000
WINDOW = 25_000      # int16 gather index range per window (2 windows/core)
CALL = 1024          # indices per dma_gather call (HW fails at 4096/call)


def _round_up(x, m):
    return (x + m - 1) // m * m


def build_nc(w_pads, rows_tbl, window, fdim=FDIM, pad_f=PAD_F, hid=HID,
             nseg=NSEG, call=CALL):
    """Build the SPMD Bass program.

    w_pads: list of per-window index counts (same on every core, multiples
    of 128).  rows_tbl: table rows per core.  window: rows per index window.
    """
    f32 = mybir.dt.float32
    i16 = mybir.dt.int16
    nwin = len(w_pads)
    ntiles = sum(w_pads) // 128

    nc = bacc.Bacc("TRN2", target_bir_lowering=False, debug=False)

    table = nc.declare_dram_parameter("table", [rows_tbl, pad_f], f32, isOutput=False)
    idxs = [
        nc.declare_dram_parameter(f"idxs{w}", [128, w_pads[w] // 16], i16, isOutput=False)
        for w in range(nwin)
    ]
    segs_d = nc.declare_dram_parameter("segs", [128, ntiles], f32, isOutput=False)
    wmat = nc.declare_dram_parameter("wmat", [fdim, hid], f32, isOutput=False)
    iota_d = nc.declare_dram_parameter("iota", [128, nseg], f32, isOutput=False)
    ident_d = nc.declare_dram_parameter("ident", [nseg, nseg], f32, isOutput=False)
    out_d = nc.declare_dram_parameter("out", [nseg, hid], f32, isOutput=True)

    with TileContext(nc) as tc:
        with (
            tc.tile_pool(name="const", bufs=1) as cpool,
            tc.tile_pool(name="gather", bufs=3) as gpool,
            tc.tile_pool(name="onehot", bufs=3) as opool,
            tc.tile_pool(name="psA", bufs=1, space="PSUM") as psA,
            tc.tile_pool(name="psT", bufs=1, space="PSUM") as psT,
            tc.tile_pool(name="sb2", bufs=1) as sb2,
        ):
            # Resident constants / index data.
            iota_t = cpool.tile([128, nseg], f32, tag="iota")
            nc.sync.dma_start(out=iota_t[:, :], in_=iota_d[:, :])
            ident_t = cpool.tile([nseg, nseg], f32, tag="ident")
            nc.sync.dma_start(out=ident_t[:, :], in_=ident_d[:, :])
            wa_t = cpool.tile([128, hid], f32, tag="wa")
            nc.sync.dma_start(out=wa_t[:, :], in_=wmat[0:128, :])
            wb_t = cpool.tile([fdim - 128, hid], f32, tag="wb")
            nc.sync.dma_start(out=wb_t[:, :], in_=wmat[128:fdim, :])
            segs_t = cpool.tile([128, ntiles], f32, tag="segs")
            nc.sync.dma_start(out=segs_t[:, :], in_=segs_d[:, :])
            idx_ts = []
            for w in range(nwin):
                it = cpool.tile([128, w_pads[w] // 16], i16, tag=f"idx{w}")
                nc.sync.dma_start(out=it[:, :], in_=idxs[w][:, :])
                idx_ts.append(it)

            a_ps = psA.tile([nseg, fdim], f32, tag="A")

            tglob = 0
            first = True
            for w in range(nwin):
                tbl_ap = table[w * window:(w + 1) * window, :]
                ncalls = _round_up(w_pads[w], call) // call
                for k in range(ncalls):
                    nidx = min(call, w_pads[w] - k * call)
                    g = nidx // 128
                    gt = gpool.tile([128, call // 128, pad_f], f32, tag="g")
                    nc.gpsimd.dma_gather(
                        out_ap=gt[:, 0:g, :],
                        in_ap=tbl_ap,
                        idxs_ap=idx_ts[w][:, k * (call // 16):
                                          k * (call // 16) + nidx // 16],
                        num_idxs=nidx,
                        num_idxs_reg=nidx,
                        elem_size=pad_f,
                    )
                    oh = opool.tile([128, call // 128, nseg], f32, tag="oh")
                    nc.vector.tensor_tensor(
                        out=oh[:, 0:g, :],
                        in0=segs_t[:, tglob:tglob + g]
                        .unsqueeze(2).broadcast_to([128, g, nseg]),
                        in1=iota_t[:, :].unsqueeze(1).broadcast_to([128, g, nseg]),
                        op=mybir.AluOpType.is_equal,
                    )
                    for j in range(g):
                        nc.tensor.matmul(
                            out=a_ps[:, :],
                            lhsT=oh[:, j, :],
                            rhs=gt[:, j, 0:fdim],
                            start=first,
                            stop=(tglob + j == ntiles - 1),
                        )
                        first = False
                    tglob += g

            # A -> SBUF, transpose to [fdim, nseg], then A @ W on-device.
            a_sb = sb2.tile([nseg, fdim], f32, tag="a_sb")
            nc.vector.tensor_copy(out=a_sb[:, :], in_=a_ps[:, :])
            t1_ps = psT.tile([128, nseg], f32, tag="t1")
            nc.tensor.transpose(out=t1_ps[:, :], in_=a_sb[:, 0:128],
                                identity=ident_t[:, :])
            at1_sb = sb2.tile([128, nseg], f32, tag="at1")
            nc.vector.tensor_copy(out=at1_sb[:, :], in_=t1_ps[:, :])
            t2_ps = psT.tile([fdim - 128, nseg], f32, tag="t2")
            nc.tensor.transpose(out=t2_ps[:, :], in_=a_sb[:, 128:fdim],
                                identity=ident_t[:, :])
            at2_sb = sb2.tile([fdim - 128, nseg], f32, tag="at2")
            nc.vector.tensor_copy(out=at2_sb[:, :], in_=t2_ps[:, :])

            o_ps = psT.tile([nseg, hid], f32, tag="o")
            nc.tensor.matmul(out=o_ps[:, :], lhsT=at1_sb[:, :], rhs=wa_t[:, :],
                             start=True, stop=False)
            nc.tensor.matmul(out=o_ps[:, :], lhsT=at2_sb[:, :], rhs=wb_t[:, :],
                             start=False, stop=True)
            o_sb = sb2.tile([nseg, hid], f32, tag="o_sb")
            nc.vector.tensor_copy(out=o_sb[:, :], in_=o_ps[:, :])
            nc.sync.dma_start(out=out_d[:, :], in_=o_sb[:, :])

    nc.compile()
    return nc


def prepare_inputs(f_atoms, W, func2atom, mapping,
                   n_cores=N_CORES, rows_tbl=ROWS_PER_CORE, window=WINDOW,
                   pad_f=PAD_F, call=CALL):
    """Shard and encode inputs for the device program.

    Returns (in_maps, w_pads, ntiles)."""
    nseg_ids = np.repeat(mapping.astype(np.int64), func2atom.shape[1])
    flat = func2atom.astype(np.int64).ravel()
    valid = flat > 0
    atom = flat[valid] - 1
    seg = nseg_ids[valid]
    core = atom // rows_tbl
    local = atom % rows_tbl
    win = local // window
    lidx = local % window
    nwin = _round_up(rows_tbl, window) // window

    # Per (core, window) counts -> shared padded sizes.
    counts = np.zeros((n_cores, nwin), dtype=np.int64)
    np.add.at(counts, (core, win), 1)
    w_pads = [int(_round_up(counts[:, w].max(), 128)) for w in range(nwin)]
    ntiles = sum(w_pads) // 128

    # Order pairs by (core, window); stable order within is irrelevant.
    order = np.lexsort((win, core))
    atom_s, seg_s, lidx_s, core_s, win_s = (
        atom[order], seg[order], lidx[order], core[order], win[order])

    in_maps = []
    f_pad = np.zeros((n_cores, rows_tbl, pad_f), dtype=np.float32)
    f_pad[:, :, :FDIM] = f_atoms.reshape(n_cores, rows_tbl, FDIM)
    iota = np.broadcast_to(np.arange(NSEG, dtype=np.float32),
                           (128, NSEG)).copy()
    ident = np.eye(NSEG, dtype=np.float32)

    for c in range(n_cores):
        msk_c = core_s == c
        im = {"table": f_pad[c], "wmat": W.astype(np.float32),
              "iota": iota, "ident": ident}
        seg_cat = []
        for w in range(nwin):
            m = msk_c & (win_s == w)
            li = lidx_s[m]
            sg = seg_s[m]
            pad = w_pads[w] - len(li)
            li = np.concatenate([li, np.zeros(pad, np.int64)])
            sg = np.concatenate([sg, -np.ones(pad, np.int64)])
            # wrap indices into 16 partitions, replicate to 128
            wrap = li.astype(np.int16).reshape(-1, 16).T  # [16, w_pad/16]
            im[f"idxs{w}"] = np.tile(wrap, (8, 1)).copy()
            seg_cat.append(sg)
        seg_cat = np.concatenate(seg_cat).astype(np.float32)
        im["segs"] = seg_cat.reshape(ntiles, 128).T.copy()
        in_maps.append(im)
    return in_maps, w_pads, ntiles


_CACHE = {}


def kernel(f_atoms, W, func2atom, mapping, func_save_init, _trace=False):
    in_maps, w_pads, ntiles = prepare_inputs(f_atoms, W, func2atom, mapping)
    key = tuple(w_pads)
    if key not in _CACHE:
        _CACHE[key] = build_nc(w_pads, ROWS_PER_CORE, WINDOW)
    nc = _CACHE[key]
    res = run_bass_kernel_spmd(nc, in_maps, list(range(N_CORES)),
                               trace=_trace)
    partial = sum(r["out"] for r in res.results)
    out = func_save_init.astype(np.float32) + partial.astype(np.float32)
    if _trace:
        kernel.last_exec_time_ns = res.exec_time_ns
    return out


# revision 10
# speedup vs baseline: 1.0134x; 1.0134x over previous
"""Trainium2 Bass kernel for CMPNEncoder functional-group embedding (v3).

out = func_save_init + A @ W,  A[m,:] = sum_a count_m[a] * f_atoms[a,:].

Per core (atoms sharded 8 ways): stream only the *referenced* atom rows
(~80% of the shard) plus their per-segment count rows, reduce via
matmuls into a PSUM [100,133] accumulator, then apply W on-device.

Precision/bandwidth trick: each f32 row is shipped as an exact hi/lo bf16
pair packed into one 532-byte DRAM row ([hi(133) | lo(133)] bf16), so DMA
moves the same bytes as f32 but the tensor engine runs bf16 matmuls
(1 cycle/row instead of 4).  A = sum w^T(hi + lo); residual error ~4e-6.
Counts are exact in bf16 and packed two 128-col tile-rows per 512-byte
DRAM row.  Host sums the per-core [100,300] partials (unshard) and adds
func_save_init.
"""

import sys

sys.path.insert(0, "/opt/trn_rl_repo")

import ml_dtypes
import numpy as np

import concourse.bacc as bacc
import concourse.mybir as mybir
from concourse.bass_utils import run_bass_kernel_spmd
from concourse.tile import TileContext

N_ATOMS = 400_000
FDIM = 133
HID = 300
NSEG = 100
N_CORES = 8
ROWS_PER_CORE = N_ATOMS // N_CORES
CHUNK = 32                                # 128-row tiles per DMA chunk


def _round_up(x, m):
    return (x + m - 1) // m * m


def build_nc(rows_pad, fdim=FDIM, hid=HID, nseg=NSEG, chunk=CHUNK):
    f32, bf16 = mybir.dt.float32, mybir.dt.bfloat16
    ntiles = rows_pad // 128
    ngrp = ntiles // 4                    # count groups (4 tiles per group)
    nchunks = (ntiles + chunk - 1) // chunk

    nc = bacc.Bacc("TRN2", target_bir_lowering=False, debug=False)

    table = nc.declare_dram_parameter("table", [rows_pad, 2 * fdim], bf16,
                                      isOutput=False)
    cnt_d = nc.declare_dram_parameter("cnt", [ngrp, 128, 512], bf16,
                                      isOutput=False)
    wmat = nc.declare_dram_parameter("wmat", [fdim, hid], f32, isOutput=False)
    ident_d = nc.declare_dram_parameter("ident", [nseg, nseg], f32,
                                        isOutput=False)
    out_d = nc.declare_dram_parameter("out", [nseg, hid], f32, isOutput=True)

    t3 = table[:, :].rearrange("(t p) f -> p t f", p=128)   # [128, nt, 266]
    c3 = cnt_d[:, :, :].transpose([1, 0, 2])                # [128, ngrp, 512]

    with TileContext(nc) as tc:
        with (
            tc.tile_pool(name="const", bufs=1) as cpool,
            tc.tile_pool(name="stream", bufs=3) as spool,
            tc.tile_pool(name="psA", bufs=1, space="PSUM") as psA,
            tc.tile_pool(name="psT", bufs=1, space="PSUM") as psT,
            tc.tile_pool(name="sb2", bufs=1) as sb2,
        ):
            ident_t = cpool.tile([nseg, nseg], f32, tag="ident")
            nc.sync.dma_start(out=ident_t[:, :], in_=ident_d[:, :])
            wa_t = cpool.tile([128, hid], f32, tag="wa")
            nc.sync.dma_start(out=wa_t[:, :], in_=wmat[0:128, :])
            wb_t = cpool.tile([fdim - 128, hid], f32, tag="wb")
            nc.sync.dma_start(out=wb_t[:, :], in_=wmat[128:fdim, :])

            a_ps = psA.tile([nseg, fdim], f32, tag="A")

            tglob = 0
            for ck in range(nchunks):
                t0 = ck * chunk
                g = min(chunk, ntiles - t0)
                ft = spool.tile([128, chunk, 2 * fdim], bf16, tag="f")
                nc.sync.dma_start(out=ft[:, 0:g, :], in_=t3[:, t0:t0 + g, :])
                wt = spool.tile([128, chunk // 4, 512], bf16, tag="w")
                nc.sync.dma_start(out=wt[:, 0:g // 4, :],
                                  in_=c3[:, t0 // 4:(t0 + g) // 4, :])
                for j in range(g):
                    lhs = wt[:, j // 4, (j % 4) * 128:(j % 4) * 128 + nseg]
                    nc.tensor.matmul(
                        out=a_ps[:, :],
                        lhsT=lhs,
                        rhs=ft[:, j, 0:fdim],
                        start=(tglob == 0),
                        stop=False,
                    )
                    nc.tensor.matmul(
                        out=a_ps[:, :],
                        lhsT=lhs,
                        rhs=ft[:, j, fdim:2 * fdim],
                        start=False,
                        stop=(tglob == ntiles - 1),
                    )
                    tglob += 1

            a_sb = sb2.tile([nseg, fdim], f32, tag="a_sb")
            nc.vector.tensor_copy(out=a_sb[:, :], in_=a_ps[:, :])
            t1_ps = psT.tile([128, nseg], f32, tag="t1")
            nc.tensor.transpose(out=t1_ps[:, :], in_=a_sb[:, 0:128],
                                identity=ident_t[:, :])
            at1_sb = sb2.tile([128, nseg], f32, tag="at1")
            nc.vector.tensor_copy(out=at1_sb[:, :], in_=t1_ps[:, :])
            t2_ps = psT.tile([fdim - 128, nseg], f32, tag="t2")
            nc.tensor.transpose(out=t2_ps[:, :], in_=a_sb[:, 128:fdim],
                                identity=ident_t[:, :])
            at2_sb = sb2.tile([fdim - 128, nseg], f32, tag="at2")
            nc.vector.tensor_copy(out=at2_sb[:, :], in_=t2_ps[:, :])

            o_ps = psT.tile([nseg, hid], f32, tag="o")
            nc.tensor.matmul(out=o_ps[:, :], lhsT=at1_sb[:, :], rhs=wa_t[:, :],
                             start=True, stop=False)
            nc.tensor.matmul(out=o_ps[:, :], lhsT=at2_sb[:, :], rhs=wb_t[:, :],
                             start=False, stop=True)
            o_sb = sb2.tile([nseg, hid], f32, tag="o_sb")
            nc.vector.tensor_copy(out=o_sb[:, :], in_=o_ps[:, :])
            nc.sync.dma_start(out=out_d[:, :], in_=o_sb[:, :])

    nc.compile()
    return nc


def prepare_inputs(f_atoms, W, func2atom, mapping,
                   n_cores=N_CORES, rows_tbl=ROWS_PER_CORE, nseg=NSEG):
    fdim = f_atoms.shape[1]
    flat = func2atom.astype(np.int64).ravel()
    seg = np.repeat(mapping.astype(np.int64), func2atom.shape[1])
    valid = flat > 0
    atom = flat[valid] - 1
    seg = seg[valid]
    core = atom // rows_tbl
    local = atom % rows_tbl

    # Per-core count matrices over the core's referenced (compacted) rows.
    percore = []
    for c in range(n_cores):
        m = core == c
        cnt = np.zeros((rows_tbl, nseg), dtype=np.float32)
        np.add.at(cnt, (local[m], seg[m]), 1.0)
        ref = np.flatnonzero(cnt.any(axis=1))
        percore.append((ref, cnt[ref]))

    rows_pad = _round_up(max(len(r) for r, _ in percore), 512)
    ntiles = rows_pad // 128
    ident = np.eye(nseg, dtype=np.float32)

    in_maps = []
    for c in range(n_cores):
        ref, cnt = percore[c]
        n = len(ref)
        rows = f_atoms[c * rows_tbl:(c + 1) * rows_tbl][ref]
        hi = rows.astype(ml_dtypes.bfloat16)
        lo = (rows - hi.astype(np.float32)).astype(ml_dtypes.bfloat16)
        tbl = np.zeros((rows_pad, 2 * fdim), dtype=ml_dtypes.bfloat16)
        tbl[:n, :fdim] = hi
        tbl[:n, fdim:] = lo
        cp = np.zeros((rows_pad, 128), dtype=ml_dtypes.bfloat16)
        cp[:n, :nseg] = cnt.astype(ml_dtypes.bfloat16)
        # pack 4 tile-rows per 1KB DRAM row: [ngrp, 128, 512] bf16
        cpk = cp.reshape(ntiles // 4, 4, 128, 128).transpose(0, 2, 1, 3) \
                .reshape(ntiles // 4, 128, 512)
        in_maps.append({
            "table": tbl,
            "cnt": np.ascontiguousarray(cpk),
            "wmat": W.astype(np.float32),
            "ident": ident,
        })
    return in_maps, rows_pad


_CACHE = {}


def kernel(f_atoms, W, func2atom, mapping, func_save_init, _trace=False):
    in_maps, rows_pad = prepare_inputs(f_atoms, W, func2atom, mapping)
    if rows_pad not in _CACHE:
        _CACHE[rows_pad] = build_nc(rows_pad)
    nc = _CACHE[rows_pad]
    res = run_bass_kernel_spmd(nc, in_maps, list(range(N_CORES)),
                               trace=_trace)
    partial = sum(r["out"] for r in res.results)
    out = func_save_init.astype(np.float32) + partial.astype(np.float32)
    if _trace:
        kernel.last_exec_time_ns = res.exec_time_ns
    return out
